# revision 6
# baseline (speedup 1.0000x reference)
"""Self-attention kernel for Trainium2, 8 NeuronCores, data-parallel over batch.

Reference computation (per batch sample, N=H*W=4096, C=64, Ck=8):
    f = x @ Wf + bf            [N, 8]
    g = x @ Wg + bg            [N, 8]
    h = x @ Wh + bh            [N, 64]
    s = f @ g^T                [N, N]
    attn = softmax(s, axis=-1)
    o = gamma * (attn @ h) + x

Kernel strategy (one sample per core):
  - Scores computed TRANSPOSED: sT[m, n] with m (the softmax-reduction index)
    on partitions.  The contraction dim is only K=9 (8 features + affine
    row), so four m-tiles' score matmuls run CONCURRENTLY in the four 32-row
    PE tile_position row groups (f/g both band-replicated across the 4
    bands).  No max subtraction (scores are O(1)); the softmax denominator
    comes free from an augmented column in h.
  - exp split across ScalarE (true exp via activation affine) and VectorE
    (fp8e4m3 Schraudolph bit-trick: i8 = max(s'/16, 0) bitcast to e4m3
    = exp(s)/8), Bresenham-interleaved over [128, 1024] PSUM chunks.
    Scores carry a C1=128*log2(e) scale and +504 offset folded into the
    weights.
  - ctx^T = [128*gamma*h | 128]^T @ exp accumulated in PSUM over m with
    fp8 DoubleRow matmuls (two m-tiles per instruction); row 64 gives
    128*sum(exp), whose reciprocal directly yields gamma*ctx.
  - Epilogue: DMA-transpose ctxT back to [n, c] layout (PE transposes for
    the final quarter), one batched reciprocal per quarter on the DVE, then
    a single fused (ctx*rden + x) scalar_tensor_tensor per n-tile on
    GpSimd so the exp engines stay dedicated to exp.
"""

import numpy as np
import ml_dtypes

import concourse.bass as bass
import concourse.mybir as mybir
import concourse.tile as tile
from concourse.bass import ts, ds
from concourse.bass_utils import run_bass_kernel_spmd
from concourse.masks import make_identity

BF16 = mybir.dt.bfloat16
FP8 = mybir.dt.float8e4
F32 = mybir.dt.float32

N = 4096          # H*W per sample
C = 64            # channels
CK = 8            # f/g projection dim
P = 128           # partitions
NT = N // P       # 32 n/m tiles
HALF = N // 2     # 2048
QW = 1024         # quarter width
NQ = N // QW      # 4
QT = QW // P      # 8 n-tiles per quarter
C1 = 128.0 * np.log2(np.e)   # score pre-scale (f side), undone by ACT affine
SCORE_OFF = 504.0            # additive score offset (exact in bf16):
                             # s'/16 = 8*log2(e)*s + 31.5, the e4m3 bit value
                             # of exp(s)/8 with the Schraudolph shift; clamped
                             # at 0 on the DVE.  ACT computes exp(s - ln 8).
                             # The /8 keeps exp in e4m3 range; softmax ratios
                             # are unaffected.

def _np_bf16(a):
    return np.ascontiguousarray(a.astype(np.float32).astype(ml_dtypes.bfloat16))


def prepare_weights(Wf, bf, Wg, bg, Wh, bh, gamma):
    """Host-side weight folding. Returns dict of bf16 arrays (dram params)."""
    Wf = np.asarray(Wf, np.float32)
    Wg = np.asarray(Wg, np.float32)
    Wh = np.asarray(Wh, np.float32)
    bf = np.asarray(bf, np.float32)
    bg = np.asarray(bg, np.float32)
    bh = np.asarray(bh, np.float32)
    gamma = float(np.asarray(gamma, np.float32))

    # f-side, scaled by C1, bias as row 8 of each band; replicated across
    # the 4 32-row bands so the 4-way row-group-packed score matmuls can
    # contract per band.  Column 32b+8 (paired with g-side column 8 == 1)
    # adds SCORE_OFF to every score so the DVE fp8 bit-trick can clamp at 0
    # instead of going negative: raw scores' = C1*s + SCORE_OFF.
    wf_aug = np.zeros((128, 128), np.float32)
    for b in range(4):
        wf_aug[:C, 32 * b: 32 * b + CK] = C1 * Wf
        wf_aug[C, 32 * b: 32 * b + CK] = C1 * bf
        wf_aug[C, 32 * b + CK] = SCORE_OFF

    # g-side, same replication, column 32b+8 = ones row
    wg_aug = np.zeros((128, 128), np.float32)
    for b in range(4):
        wg_aug[:C, 32 * b: 32 * b + CK] = Wg
        wg_aug[C, 32 * b: 32 * b + CK] = bg
        wg_aug[C, 32 * b + CK] = 1.0

    # h-side scaled by 128*gamma (keeps fp8 h out of subnormals) and a
    # 128-valued denominator column at 64; the epilogue's reciprocal of
    # 128*sum(exp) then yields gamma*ctx directly: [128, 128]
    wh_aug = np.zeros((128, 128), np.float32)
    wh_aug[:C, :C] = 128.0 * gamma * Wh
    wh_aug[C, :C] = 128.0 * gamma * bh
    wh_aug[C, C] = 128.0

    return {
        "wf": _np_bf16(wf_aug),
        "wg": _np_bf16(wg_aug),
        "wh": _np_bf16(wh_aug),
    }


def _spill_excess_waits(nc, limit=1):
    """Walrus rejects HW-queue instructions carrying more than a couple of
    semaphore waits.  Move excess waits onto standalone EventSemaphore
    instructions inserted just before the offender on the same engine
    (cumulative sem-ge waits split across instructions are equivalent)."""
    n_spill = 0
    for bb in nc.main_func.blocks:
        rebuilt = []
        changed = False
        for ins in bb.instructions:
            si = ins.sync_info
            if si is not None and len(si.on_wait) > limit:
                waits = list(si.on_wait)
                for w in waits[limit:]:
                    ev = mybir.InstEventSemaphore(
                        name=f"wspill-{n_spill}", ins=[], outs=[])
                    ev.engine = ins.engine
                    ev.sync_info = mybir.SyncInfo(on_wait=[w], on_update=[])
                    rebuilt.append(ev)
                    n_spill += 1
                ins.sync_info = mybir.SyncInfo(
                    on_wait=waits[:limit], on_update=list(si.on_update))
                changed = True
            rebuilt.append(ins)
        if changed:
            bb.instructions = rebuilt
    return n_spill


def _dedup_ldweights(nc):
    """Drop an InstLdweights whose weight AP/mode is identical to the
    immediately preceding LDW on the PE queue (score j-chunk and DoubleRow
    pairs reuse the same stationary operand).  Only sync-free LDWs are
    dropped so no semaphore edges are lost."""
    n_drop = 0
    for bb in nc.main_func.blocks:
        rebuilt = []
        last_key = None
        changed = False
        for ins in bb.instructions:
            tname = type(ins).__name__
            if tname == "InstLdweights":
                si = ins.sync_info
                clean = si is None or (not si.on_wait and not si.on_update)
                key = (str(ins.ins[0]), str(getattr(ins, "perf_mode", None)),
                       str(getattr(ins, "tile_position", None)),
                       str(getattr(ins, "is_transpose", None)))
                if clean and key == last_key:
                    n_drop += 1
                    changed = True
                    continue
                last_key = key
            elif tname == "InstMatmult":
                pass  # matmul leaves the stationary operand in place
            elif ins.engine == mybir.EngineType.PE:
                last_key = None
            rebuilt.append(ins)
        if changed:
            bb.instructions = rebuilt
    return n_drop


def build_bass(repeat=1, spill=True):
    """Build the per-core Bass graph (SPMD: same graph on all 8 cores).
    repeat > 1 duplicates the whole body for timing calibration."""
    nc = bass.Bass()

    x_d = nc.declare_dram_parameter("x", [P, NT * C], F32, isOutput=False)
    wf_d = nc.declare_dram_parameter("wf", [128, 128], BF16, isOutput=False)
    wg_d = nc.declare_dram_parameter("wg", [128, 128], BF16, isOutput=False)
    wh_d = nc.declare_dram_parameter("wh", [128, 128], BF16, isOutput=False)
    xt_d = nc.declare_dram_parameter("xta", [65, N], BF16, isOutput=False)
    out_d = nc.declare_dram_parameter("out", [N, C], F32, isOutput=True)

    with tile.TileContext(nc) as tc:
        for _ in range(repeat):
            _build_body(nc, tc, x_d, wf_d, wg_d, wh_d, xt_d, out_d)
    _dedup_ldweights(nc)
    if spill:
        _spill_excess_waits(nc)
    return nc


def _build_body(nc, tc, x_d, wf_d, wg_d, wh_d, xt_d, out_d):
    from contextlib import ExitStack

    with ExitStack() as ctx:
        consts = ctx.enter_context(tc.tile_pool(name="consts", bufs=1))
        exp_pool = ctx.enter_context(tc.tile_pool(name="expp", bufs=3))
        work = ctx.enter_context(tc.tile_pool(name="work", bufs=6))

        # ---- load x (host pre-tiled to [p, t*c] f32): contiguous DMAs ----
        # sync (HWDGE) + gpsimd (SWDGE) queues only: scalar stays free for exp
        x_sb = consts.tile([P, NT, C], F32)
        x3 = x_d.rearrange("p (t c) -> p t c", c=C)
        for d in range(4):
            (nc.sync if d % 2 == 0 else nc.gpsimd).dma_start(
                x_sb[:, ds(4 * d, 4), :], x3[:, ds(4 * d, 4), :])
        # ---- constants (small, after x on the queues) ----
        wf_sb = consts.tile([128, 128], BF16)
        wg_sb = consts.tile([128, 128], BF16)
        wh_sb = consts.tile([128, 128], BF16)
        nc.sync.dma_start(wf_sb[:], wf_d[:])
        nc.gpsimd.dma_start(wg_sb[:], wg_d[:])
        nc.sync.dma_start(wh_sb[:], wh_d[:])

        # identity for the final-quarter PE transposes (PE is idle then)
        id_sb = consts.tile([128, 128], BF16)
        make_identity(nc, id_sb[:])

        # --- head warmup: engines are otherwise idle for the NEFF startup +
        # input DMA.  Pull the ScalarE exp table load (~2.7us) and the PE HAM
        # un-throttle (~3.4us of sustained activity) into that window.
        warm = consts.tile([128, 512], BF16)
        nc.vector.memset(warm[:], 0.0)
        wtmp = consts.tile([128, 8], BF16)
        nc.scalar.activation(wtmp[:], warm[:, :8],
                             mybir.ActivationFunctionType.Exp,
                             bias=0.0, scale=1.0)
        with tc.tile_pool(name="warm_ps", bufs=1, space="PSUM") as warm_ps:
            wp = warm_ps.tile([128, 512], F32)
            for _ in range(20):
                nc.tensor.matmul(wp[:], warm[:, :128], warm[:],
                                 start=True, stop=True)

        # ACT exp bias: exp(s'/C1 + bias) = exp(s - ln 8)
        ebias = consts.tile([P, 1], F32)
        nc.vector.memset(ebias[:], float(-SCORE_OFF / C1 - np.log(8.0)))

        # ---- xT_aug [128, N] bf16: rows 0..64 host-built [x^T ; ones],
        # rows 65..127 zeroed on device ----
        xt_sb = consts.tile([128, N], BF16)
        nc.vector.memset(xt_sb[C:, :], 0.0)
        for d in range(2):
            (nc.sync if d == 0 else nc.gpsimd).dma_start(
                xt_sb[:65, ds(d * HALF, HALF)], xt_d[:, ds(d * HALF, HALF)])

        with tc.tile_pool(name="pro_ps", bufs=3, space="PSUM") as pro_ps:
            # f/g projections (f scaled by C1), band-replicated.  Emission
            # order front-loads exactly what main-loop group (q0, t) needs:
            # f chunks 0-1 (n 0:1024), then per-t g chunk + h group.
            f_sb = consts.tile([128, N], BF16)
            g_sb = consts.tile([128, N], BF16)
            h_sb = consts.tile([P, NT, 128], FP8)

            def emit_f(chunk):
                pf = pro_ps.tile([128, 512], F32, tag="fg", name="pf")
                nc.tensor.matmul(pf[:], wf_sb[:, :], xt_sb[:, ts(chunk, 512)],
                                 start=True, stop=True)
                nc.any.tensor_copy(f_sb[:, ts(chunk, 512)], pf[:])

            def emit_g(chunk):
                pg = pro_ps.tile([128, 512], F32, tag="fg", name="pg")
                nc.tensor.matmul(pg[:], wg_sb[:, :], xt_sb[:, ts(chunk, 512)],
                                 start=True, stop=True)
                nc.any.tensor_copy(g_sb[:, ts(chunk, 512)], pg[:])

            def emit_h(grp):
                ph = pro_ps.tile([128, 512], F32, tag="fg", name="ph")
                for j in range(4):
                    m = 4 * grp + j
                    nc.tensor.matmul(ph[:, ts(j, P)], xt_sb[:, ts(m, P)],
                                     wh_sb[:], start=True, stop=True)
                nc.any.tensor_copy(h_sb[:, ds(4 * grp, 4), :], ph[:])

            emit_f(0)
            emit_f(1)
            for grp in range(8):
                emit_g(grp)
                emit_h(grp)
            for chunk in range(2, 8):
                emit_f(chunk)

        # x tiles 16..31 (quarters 2-3 residuals, needed late): behind
        # the compute-critical loads on each queue
        for d in range(4, 8):
            (nc.sync if d % 2 == 0 else nc.gpsimd).dma_start(
                x_sb[:, ds(4 * d, 4), :], x3[:, ds(4 * d, 4), :])

        # ---- main: scores -> exp -> ctxT accumulate; epilogue, per quarter.
        # Score matmuls are 4-way row-group packed: band b (rows 32b..32b+31)
        # computes m-tile 4t+b.  Each (q, t) group produces four [128, 2, 512]
        # PSUM pair-tiles (2 banks each; pool of 3 + ctx 2 banks = 8 banks).
        with tc.tile_pool(name="ps_s", bufs=3, space="PSUM") as ps_s, \
             tc.tile_pool(name="ps_ctx", bufs=1, space="PSUM") as ps_ctx:
            # exp engine assignment: ACT chunk ~997ns vs DVE ~1192ns -> give
            # ACT ~70 of 128 chunks, spread evenly (Bresenham).
            N_CHUNKS = 128
            DVE_SHARE = 54
            use_dve = [((i * DVE_SHARE) % N_CHUNKS) < DVE_SHARE
                       for i in range(N_CHUNKS)]
            chunk_idx = 0

            for q in range(NQ):
                ctx_ps = ps_ctx.tile([128, QW], F32, tag="ctx")
                for t in range(8):
                    e_q = exp_pool.tile([128, 4, QW], FP8, tag="e")
                    sp = {}
                    for pr in range(2):
                        for j in range(2):
                            sp[(pr, j)] = ps_s.tile(
                                [128, 2, 512], F32, tag="s", name=f"sp{pr}{j}")
                    # 8 score matmuls: band-major so each band's pair of
                    # j-chunks shares one [32, 128] LDW (deduped), while the
                    # 4 bands run concurrently in their row groups.
                    for b in range(4):
                        m = 4 * t + b
                        for j in range(2):
                            nc.tensor.matmul(
                                sp[(b // 2, j)][:, b % 2, :],
                                g_sb[ds(32 * b, 32), ts(m, P)],
                                f_sb[ds(32 * b, 32), ds(q * QW + j * 512, 512)],
                                start=True, stop=True,
                                tile_position=(32 * b, 0))
                    # exp: 4 [128, 1024] chunks, each one PSUM pair-tile
                    for pr in range(2):
                        for j in range(2):
                            spt = sp[(pr, j)]
                            if use_dve[chunk_idx]:
                                _dve_exp(nc, e_q, pr, j, spt)
                            else:
                                nc.scalar.activation(
                                    e_q[:, ds(2 * pr, 2), ds(j * 512, 512)],
                                    spt[:],
                                    mybir.ActivationFunctionType.Exp,
                                    bias=ebias[:], scale=float(1.0 / C1))
                            chunk_idx += 1
                    # ctx accumulate: pair-major so each h pair's two j-chunk
                    # matmuls share one DoubleRow LDW (deduped)
                    for pr in range(2):
                        for j in range(2):
                            nc.tensor.matmul(
                                ctx_ps[:, ds(j * 512, 512)],
                                h_sb[:, ds(4 * t + 2 * pr, 2), :],
                                e_q[:, ds(2 * pr, 2), ds(j * 512, 512)],
                                perf_mode=mybir.MatmulPerfMode.DoubleRow,
                                start=(t == 0 and pr == 0),
                                stop=(t == 7 and pr == 1))

                # epilogue for this quarter: copy ctxT to SBUF bf16 (halves so
                # transposes start before the full copy), transpose back to
                # [n, c], batched reciprocal, fused scale+residual on GpSimd.
                ctxt_sb = work.tile([128, QW], BF16, tag="ctxt")
                nc.any.tensor_copy(ctxt_sb[:, :QW // 2], ctx_ps[:, :QW // 2])
                nc.any.tensor_copy(ctxt_sb[:, QW // 2:], ctx_ps[:, QW // 2:])
                last_q = q == NQ - 1
                rden = work.tile([P, QT], F32, tag="rden")
                if last_q:
                    # nothing left for the PE: transpose on it instead of the
                    # DMA xbar so the tail isn't queue-serialized
                    tr_ps = ps_s.tile([128, QW], BF16, tag="s", name="trps")
                    for t2 in range(QT):
                        nc.tensor.transpose(tr_ps[:, ts(t2, P)],
                                            ctxt_sb[:, ts(t2, P)], id_sb[:])
                    tr3 = tr_ps.rearrange("p (t c) -> p t c", c=P)
                    nc.vector.reciprocal(rden[:], tr3[:, :, C: C + 1])
                else:
                    o_tr = work.tile([128, QT, P], BF16, tag="otr")
                    for t2 in range(QT):
                        nc.sync.dma_start_transpose(
                            o_tr[:, t2, :], ctxt_sb[:, ts(t2, P)])
                    nc.vector.reciprocal(rden[:], o_tr[:, :, C: C + 1])
                for t2 in range(QT):
                    osb = work.tile([P, C], F32, tag="osb")
                    blk = tr3[:, t2, :C] if last_q else o_tr[:, t2, :C]
                    nc.vector.scalar_tensor_tensor(
                        osb[:], blk, rden[:, ds(t2, 1)],
                        x_sb[:, q * QT + t2, :],
                        mybir.AluOpType.mult, mybir.AluOpType.add)
                    (nc.gpsimd if t2 % 2 == 0 else nc.sync).dma_start(
                        out_d[ds((q * QT + t2) * P, P), :], osb[:])


def _dve_exp(nc, e_q, pr, j, s_ps):
    """fp8e4m3 bit-trick exp on the DVE: i8 = round(max(s'/16, 0))
    reinterpreted as e4m3 ~= exp(s)/8.  s' = C1*s + SCORE_OFF (from the
    weights), so s'/16 = 8*log2(e)*s + 31.5 -- the e4m3 bit pattern of
    exp(s)/8; ultra-negative scores clamp to +0."""
    i8_view = e_q.bitcast(mybir.dt.int8)
    nc.vector.tensor_scalar(i8_view[:, ds(2 * pr, 2), ds(j * 512, 512)],
                            s_ps[:], 1.0 / 16.0, 0.0,
                            mybir.AluOpType.mult, mybir.AluOpType.max)


_CACHE = {}


def _get_nc():
    if "nc" not in _CACHE:
        _CACHE["nc"] = build_bass()
    return _CACHE["nc"]


def kernel(x, Wf, bf, Wg, bg, Wh, bh, gamma):
    x = np.asarray(x, np.float32)
    B = x.shape[0]
    assert x.shape == (B, 64, 64, 64) and B == 8

    w = prepare_weights(Wf, bf, Wg, bg, Wh, bh, gamma)
    nc = _get_nc()
    xt = x.reshape(B, NT, P, C).transpose(0, 2, 1, 3).reshape(B, P, NT * C)
    xta = np.ones((B, 65, N), np.float32)
    xta[:, :C, :] = x.reshape(B, N, C).transpose(0, 2, 1)
    xta = xta.astype(ml_dtypes.bfloat16)
    in_maps = [{"x": np.ascontiguousarray(xt[i]),
                "xta": np.ascontiguousarray(xta[i]), **w} for i in range(B)]
    res = run_bass_kernel_spmd(nc, in_maps, core_ids=list(range(8)))
    out = np.stack([np.asarray(res.results[i]["out"]).reshape(64, 64, 64)
                    for i in range(B)])
    return out.astype(np.float32)
